# revision 17
# baseline (speedup 1.0000x reference)
"""Multi-head attention (B=2, S=2048, D=1024, H=16) on 8 trn2 NeuronCores.

Sharding: core = (batch b, head-group g of 4 heads); 2 batches x 4 groups.
Each core computes, for its batch and its 4 heads:
  - transposed projections QT/KT/VT = W @ x^T (fp32r matmuls),
  - pass B ([k,q] orientation): S^T = K Q^T, exp, mask, attn^T @ V via
    V augmented with a ones-row so the softmax denominator falls out of
    the same matmul; context normalized on-chip,
  - pass A ([q,k] orientation): S = Q K^T, exp with bias=-ln(den) per
    partition (q) -> normalized attention probabilities, masked, DMA'd out,
  - partial out-projection for its 256 context dims (host sums over g).
Host: input/weight transposes, mask conversion, output assembly.
"""

import os
import numpy as np
from contextlib import ExitStack

import concourse.bass as bass
import concourse.tile as tile
import concourse.mybir as mybir
from concourse.masks import make_identity

F32 = mybir.dt.float32
F32R = mybir.dt.float32r
BF16 = mybir.dt.bfloat16
U8 = mybir.dt.uint8
EXP = mybir.ActivationFunctionType.Exp
LN = mybir.ActivationFunctionType.Ln

B, S, D = 2, 2048, 1024
H = 16
DK = 64  # head dim
HPC = 4  # heads per core
DC = HPC * DK  # context dims per core (256)
NC_CORES = 8
KC = S // 128  # 16 k/q chunks
SCALE = 1.0 / np.sqrt(DK)

MASK_DT = U8  # mask storage dtype on device
GPB = 10  # of 16 pass-B mask-mults per round, how many on GPSIMD (rest DVE)
GPB0 = 6  # same, for round 0 (DVE idle there)




def build_program():
    nc = bass.Bass(
        "TRN2", target_bir_lowering=False, debug=False, enable_partition_id=False
    )
    # --- DRAM tensors (per-core views; same program on all 8 cores) ---
    xT = {
        n: nc.dram_tensor(f"{n}T", [D, S], F32R, kind="ExternalInput")
        for n in ("q", "k", "v")
    }
    wT = {
        n: nc.dram_tensor(f"W{n}T", [D, DC], F32R, kind="ExternalInput")
        for n in ("q", "k", "v")
    }
    bias_d = {
        n: nc.dram_tensor(f"b{n}s", [2, 128, 1], F32, kind="ExternalInput")
        for n in ("q", "k", "v")
    }
    woT = nc.dram_tensor("WoTs", [128, 2, D], F32R, kind="ExternalInput")
    m01 = nc.dram_tensor("m01", [S, S], MASK_DT, kind="ExternalInput")
    m01T = nc.dram_tensor("m01T", [S, S], MASK_DT, kind="ExternalInput")
    attn4 = nc.dram_tensor("attn4", [HPC, S, S], F32, kind="ExternalOutput")
    outp = nc.dram_tensor("outp", [S, D], F32, kind="ExternalOutput")

    with ExitStack() as ctx:
        tc = ctx.enter_context(tile.TileContext(nc))

        consts = ctx.enter_context(tc.tile_pool(name="consts", bufs=1))
        ident = consts.tile([128, 128], F32)
        make_identity(nc, ident)
        ones64 = consts.tile([1, 64], F32)
        nc.gpsimd.memset(ones64, 1.0)
        bias_sb = {}
        for n in ("q", "k", "v"):
            t = consts.tile([128, 2], F32, tag=f"bias{n}", name=f"bias{n}")
            for p in range(2):
                nc.sync.dma_start(out=t[:, p : p + 1], in_=bias_d[n][p])
            bias_sb[n] = t

        # masks resident, loaded first so they overlap stage-1 DMA
        maskpool = ctx.enter_context(tc.tile_pool(name="masks", bufs=1))
        mA = maskpool.tile([128, KC, S], MASK_DT, tag="mA")
        nc.sync.dma_start(out=mA, in_=m01.rearrange("(c p) k -> p c k", p=128))
        mB = maskpool.tile([128, KC, S], MASK_DT, tag="mB")
        nc.sync.dma_start(out=mB, in_=m01T.rearrange("(c p) q -> p c q", p=128))

        # Resident activations
        resident = ctx.enter_context(tc.tile_pool(name="resident", bufs=1))
        QT = resident.tile([128, 2, S], F32R, tag="QT")
        KT = resident.tile([128, 2, S], F32R, tag="KT")
        ctxAll = resident.tile([128, 2, S], F32R, tag="ctxAll")
        vaug = [
            resident.tile([128, KC, DK + 1], BF16, tag=f"vaug{h}", name=f"vaug{h}") for h in range(HPC)
        ]
        negln = [
            resident.tile([128, KC], F32, tag=f"negln{h}", name=f"negln{h}") for h in range(HPC)
        ]

        # ---- Stage 1: projections QT/KT/VT = W @ xT (+bias) ----
        with tc.tile_pool(name="vt", bufs=1) as vtpool:
            VT = vtpool.tile([128, 2, S], F32, tag="VT")
            dests = {"q": QT, "k": KT, "v": VT}
            with (
                tc.tile_pool(name="w", bufs=1) as wpool,
                tc.tile_pool(name="x", bufs=2) as xpool,
                tc.tile_pool(name="psproj", bufs=2, space="PSUM") as psproj,
            ):
                w_sb = {}
                for n in ("q", "k", "v"):
                    w_sb[n] = wpool.tile([128, 8, DC], F32R, tag=f"w{n}", name=f"w{n}")
                    nc.sync.dma_start(
                        out=w_sb[n], in_=wT[n].rearrange("(c p) n -> p c n", p=128)
                    )
                for n in ("q", "k", "v"):  # q,k first: lets pass-B S^T/exp start early
                    ps = [psproj.tile([128, S], F32, tag="proj", name=f"psproj{n}{i}") for i in range(2)]
                    for dm in range(8):
                        xt = xpool.tile([128, S], F32R, tag="xt")
                        nc.sync.dma_start(
                            out=xt, in_=xT[n].rearrange("(c p) s -> c p s", p=128)[dm]
                        )
                        for p in range(2):
                            for j in range(4):
                                nc.tensor.matmul(
                                    ps[p][:, 512 * j : 512 * (j + 1)],
                                    w_sb[n][:, dm, 128 * p : 128 * (p + 1)],
                                    xt[:, 512 * j : 512 * (j + 1)],
                                    start=(dm == 0),
                                    stop=(dm == 7),
                                )
                    for p in range(2):
                        nc.vector.tensor_scalar_add(
                            dests[n][:, p, :], ps[p], bias_sb[n][:, p : p + 1]
                        )

            # ---- Stage 2: V_aug (V natural layout, bf16, +ones col) ----
            with tc.tile_pool(name="psv", bufs=4, space="PSUM") as psv:
                for h in range(HPC):
                    p, rr = h // 2, 64 * (h % 2)
                    nc.gpsimd.memset(vaug[h][:, :, DK : DK + 1], 1.0)
                    for c in range(KC):
                        pst = psv.tile([128, DK], F32, tag="vtr")
                        nc.tensor.transpose(
                            pst,
                            VT[rr : rr + 64, p, 128 * c : 128 * (c + 1)],
                            ident[rr : rr + 64, rr : rr + 64],
                        )
                        nc.vector.tensor_copy(vaug[h][:, c, 0:DK], pst)

        # ---- Attention, chunk-interleaved: round h emits pass-B chunks of
        # head h (S^T -> exp -> mask -> ctx^T accum) alternating with pass-A
        # chunks of head h-1 (S -> exp(bias=-ln den) -> mask -> DMA out), so
        # ACT/DMA/DVE/GPSIMD all stay busy. PSUM: ctx [65,2048] (4 banks) +
        # shared scores pool 2x[128,1024] (4 banks) = 8.
        with (
            tc.tile_pool(name="expb", bufs=3) as expb,
            tc.tile_pool(name="smallb", bufs=2) as smallb,
            tc.tile_pool(name="attn", bufs=3) as attnp,
            tc.tile_pool(name="psctx", bufs=1, space="PSUM") as psctx,
            tc.tile_pool(name="pssc", bufs=3, space="PSUM") as pssc,
        ):
            mult_ctr = 0

            def b_half(h, ha, c, ps_ctx):
                nonlocal mult_ctr
                p, rr = h // 2, 64 * (h % 2)
                q0 = 1024 * ha
                ps_s = pssc.tile(
                    [128, 1024], F32, tag="sc", name=f"psB{h}_{c}_{ha}"
                )
                for j in range(2):
                    nc.tensor.matmul(
                        ps_s[:, 512 * j : 512 * (j + 1)],
                        KT[rr : rr + 64, p, 128 * c : 128 * (c + 1)],
                        QT[rr : rr + 64, p, q0 + 512 * j : q0 + 512 * (j + 1)],
                        start=True,
                        stop=True,
                    )
                et = expb.tile([128, 1024], BF16, tag="exp", name="et")
                nc.scalar.activation(et, ps_s, EXP, scale=float(SCALE))
                em = expb.tile([128, 1024], BF16, tag="expm", name="em")
                mult_ctr += 1
                gpb = GPB0 if h == 0 else GPB
                eng = nc.gpsimd if (mult_ctr % 16) < gpb else nc.vector
                eng.tensor_mul(em, et, mB[:, c, q0 : q0 + 1024])
                for j in range(2):
                    nc.tensor.matmul(
                        ps_ctx[:, 512 * j : 512 * (j + 1)],
                        vaug[h][:, c, :],
                        em[:, 512 * j : 512 * (j + 1)],
                        start=(c == 0),
                        stop=(c == KC - 1),
                    )

            def b_tail_half(h, ha, ps_ctx):
                """Denominators + ctx normalization for head h, q-half ha."""
                p, rr = h // 2, 64 * (h % 2)
                q0 = 1024 * ha
                recip = smallb.tile([1, 1024], F32, tag="recip", name="recip")
                nc.vector.reciprocal(recip, ps_ctx[64:65, :])
                nlog = smallb.tile([1, 1024], F32, tag="nlog", name="nlog")
                nc.scalar.activation(nlog, recip, LN)
                ps_b = pssc.tile([128, 8], F32, tag="sc", name=f"psbias{h}{ha}")
                for qc in range(8):
                    nc.tensor.transpose(
                        ps_b[:, qc : qc + 1],
                        nlog[0:1, 128 * qc : 128 * (qc + 1)],
                        ident[0:1, 0:1],
                    )
                nc.vector.tensor_copy(negln[h][:, 8 * ha : 8 * ha + 8], ps_b)
                ps_rb = pssc.tile([64, 1024], F32, tag="sc", name=f"psrb{h}{ha}")
                for j in range(2):
                    nc.tensor.matmul(
                        ps_rb[:, 512 * j : 512 * (j + 1)],
                        ones64,
                        recip[:, 512 * j : 512 * (j + 1)],
                        start=True,
                        stop=True,
                    )
                rb = smallb.tile([64, 1024], F32, tag="rb", name="rb")
                nc.vector.tensor_copy(rb, ps_rb)
                nc.vector.tensor_mul(
                    ctxAll[rr : rr + 64, p, q0 : q0 + 1024],
                    ps_ctx[0:64, :],
                    rb,
                )

            def a_half(h, qc, ha):
                p, rr = h // 2, 64 * (h % 2)
                k0 = 1024 * ha
                ps_s = pssc.tile(
                    [128, 1024], F32, tag="sc", name=f"psA{h}_{qc}_{ha}"
                )
                for j in range(2):
                    nc.tensor.matmul(
                        ps_s[:, 512 * j : 512 * (j + 1)],
                        QT[rr : rr + 64, p, 128 * qc : 128 * (qc + 1)],
                        KT[rr : rr + 64, p, k0 + 512 * j : k0 + 512 * (j + 1)],
                        start=True,
                        stop=True,
                    )
                ae = attnp.tile([128, 1024], F32, tag="ae", name="ae")
                nc.scalar.activation(
                    ae, ps_s, EXP, scale=float(SCALE),
                    bias=negln[h][:, qc : qc + 1],
                )
                ao = attnp.tile([128, 1024], F32, tag="ao", name="ao")
                nc.vector.tensor_mul(ao, ae, mA[:, qc, k0 : k0 + 1024])
                nc.sync.dma_start(
                    out=attn4[h, 128 * qc : 128 * (qc + 1), k0 : k0 + 1024],
                    in_=ao,
                )

            for h in range(HPC + 1):
                for ha in range(2):
                    ps_ctx = (
                        psctx.tile(
                            [65, 1024], F32, tag="ctx", name=f"ctx{h}{ha}"
                        )
                        if h < HPC
                        else None
                    )
                    for c in range(KC):
                        if h < HPC:
                            b_half(h, ha, c, ps_ctx)
                        if h > 0:
                            i = ha * KC + c
                            a_half(h - 1, i // 2, i % 2)
                    if h < HPC:
                        b_tail_half(h, ha, ps_ctx)

        # ---- Out-projection (partial over this core's 256 ctx dims) ----
        with (
            tc.tile_pool(name="wo", bufs=1) as wop,
            tc.tile_pool(name="outs", bufs=3) as outsp,
            tc.tile_pool(name="pso", bufs=2, space="PSUM") as pso,
        ):
            wo_sb = wop.tile([128, 2, D], F32R, tag="wo")
            nc.sync.dma_start(out=wo_sb, in_=woT.ap())
            for sc in range(KC):
                ps_o = pso.tile([128, D], F32, tag="o")
                for p in range(2):
                    for j in range(2):
                        nc.tensor.matmul(
                            ps_o[:, 512 * j : 512 * (j + 1)],
                            ctxAll[:, p, 128 * sc : 128 * (sc + 1)],
                            wo_sb[:, p, 512 * j : 512 * (j + 1)],
                            start=(p == 0),
                            stop=(p == 1),
                        )
                ot = outsp.tile([128, D], F32, tag="ot")
                nc.vector.tensor_copy(ot, ps_o)
                nc.sync.dma_start(
                    out=outp[128 * sc : 128 * (sc + 1), :], in_=ot
                )

    from bir_fixups import split_waits

    split_waits(nc)
    return nc


def prep_inputs(query, key, value, mask, Wq, bq, Wk, bk, Wv, bv, Wo, bo):
    """Host-side preprocessing -> list of 8 per-core input maps."""
    mask_np = np.asarray(mask)
    m01_np = [
        np.ascontiguousarray(mask_np[b, 0]).astype(np.uint8) for b in range(B)
    ]
    m01T_np = [np.ascontiguousarray(m.T) for m in m01_np]
    xT_np = {
        "qT": [np.ascontiguousarray(np.asarray(query)[b].T) for b in range(B)],
        "kT": [np.ascontiguousarray(np.asarray(key)[b].T) for b in range(B)],
        "vT": [np.ascontiguousarray(np.asarray(value)[b].T) for b in range(B)],
    }
    W = {"q": np.asarray(Wq), "k": np.asarray(Wk), "v": np.asarray(Wv)}
    bvec = {"q": np.asarray(bq), "k": np.asarray(bk), "v": np.asarray(bv)}
    Wo_np = np.asarray(Wo)
    in_maps = []
    for core in range(NC_CORES):
        b, g = core // 4, core % 4
        rows = slice(DC * g, DC * (g + 1))
        m = {
            "qT": xT_np["qT"][b],
            "kT": xT_np["kT"][b],
            "vT": xT_np["vT"][b],
            "m01": m01_np[b],
            "m01T": m01T_np[b],
            "WoTs": np.ascontiguousarray(
                Wo_np[:, rows].T.reshape(2, 128, D).transpose(1, 0, 2)
            ),
        }
        for n in ("q", "k", "v"):
            m[f"W{n}T"] = np.ascontiguousarray(W[n][rows, :].T)
            m[f"b{n}s"] = np.ascontiguousarray(
                bvec[n][rows].reshape(2, 128, 1).astype(np.float32)
            )
        in_maps.append(m)
    return in_maps


def assemble(results, bo):
    """results: list of 8 per-core output dicts -> (output, attn)."""
    attn = np.empty((B, H, S, S), np.float32)
    output = np.zeros((B, S, D), np.float32)
    for core in range(NC_CORES):
        b, g = core // 4, core % 4
        attn[b, HPC * g : HPC * (g + 1)] = results[core]["attn4"]
        output[b] += results[core]["outp"]
    output += np.asarray(bo)[None, None, :]
    return output, attn


_CACHE = {}


def kernel(**inputs):
    import sys

    sys.path.insert(0, os.path.dirname(os.path.abspath(__file__)))
    if "runner" not in _CACHE:
        from run_utils import SpmdRunner

        nc = build_program()
        _CACHE["runner"] = SpmdRunner(nc, NC_CORES)
    runner = _CACHE["runner"]
    in_maps = prep_inputs(**inputs)
    dev_in = runner.device_inputs(in_maps)
    results = runner.run(dev_in)
    return assemble(results, inputs["bo"])


if __name__ == "__main__":
    nc = build_program()
    print("built ok; instructions:", sum(len(bb.instructions) for f in nc.m.functions for bb in f.blocks))


# revision 19
# speedup vs baseline: 2.0934x; 2.0934x over previous
"""Multi-head attention (B=2, S=2048, D=1024, H=16) on 8 trn2 NeuronCores.

Sharding: core = (batch b, head-group g of 4 heads); 2 batches x 4 groups.
Each core computes, for its batch and its 4 heads:
  - transposed projections QT/KT/VT = W @ x^T (fp32r matmuls),
  - pass B ([k,q] orientation): S^T = K Q^T, exp, mask, attn^T @ V via
    V augmented with a ones-row so the softmax denominator falls out of
    the same matmul; context normalized on-chip,
  - pass A ([q,k] orientation): S = Q K^T, exp with bias=-ln(den) per
    partition (q) -> normalized attention probabilities, masked, DMA'd out,
  - partial out-projection for its 256 context dims (host sums over g).
Host: input/weight transposes, mask conversion, output assembly.
"""

import os
import numpy as np
from contextlib import ExitStack

import concourse.bass as bass
import concourse.tile as tile
import concourse.mybir as mybir
from concourse.masks import make_identity

F32 = mybir.dt.float32
F32R = mybir.dt.float32r
BF16 = mybir.dt.bfloat16
U8 = mybir.dt.uint8
EXP = mybir.ActivationFunctionType.Exp
LN = mybir.ActivationFunctionType.Ln

B, S, D = 2, 2048, 1024
H = 16
DK = 64  # head dim
HPC = 4  # heads per core
DC = HPC * DK  # context dims per core (256)
NC_CORES = 8
KC = S // 128  # 16 k/q chunks
SCALE = 1.0 / np.sqrt(DK)

MASK_DT = U8  # mask storage dtype on device
GPB = 10  # of 16 pass-B mask-mults per round, how many on GPSIMD (rest DVE)
GPB0 = 6  # same, for round 0 (DVE idle there)




def build_program():
    nc = bass.Bass(
        "TRN2", target_bir_lowering=False, debug=False, enable_partition_id=False
    )
    # --- DRAM tensors (per-core views; same program on all 8 cores) ---
    xT = {
        n: nc.dram_tensor(f"{n}T", [D, S], F32R, kind="ExternalInput")
        for n in ("q", "k", "v")
    }
    wT = {
        n: nc.dram_tensor(f"W{n}T", [D, DC], F32R, kind="ExternalInput")
        for n in ("q", "k", "v")
    }
    bias_d = {
        n: nc.dram_tensor(f"b{n}s", [2, 128, 1], F32, kind="ExternalInput")
        for n in ("q", "k", "v")
    }
    woT = nc.dram_tensor("WoTs", [128, 2, D], F32R, kind="ExternalInput")
    m01 = nc.dram_tensor("m01", [S, S], MASK_DT, kind="ExternalInput")
    m01T = nc.dram_tensor("m01T", [S, S], MASK_DT, kind="ExternalInput")
    attn4 = nc.dram_tensor("attn4", [HPC, S, S], F32, kind="ExternalOutput")
    outp = nc.dram_tensor("outp", [S, D], F32, kind="ExternalOutput")

    with ExitStack() as ctx:
        tc = ctx.enter_context(tile.TileContext(nc))

        consts = ctx.enter_context(tc.tile_pool(name="consts", bufs=1))
        ident = consts.tile([128, 128], F32)
        make_identity(nc, ident)
        ones64 = consts.tile([1, 64], F32)
        nc.gpsimd.memset(ones64, 1.0)
        bias_sb = {}
        for n in ("q", "k", "v"):
            t = consts.tile([128, 2], F32, tag=f"bias{n}", name=f"bias{n}")
            for p in range(2):
                nc.sync.dma_start(out=t[:, p : p + 1], in_=bias_d[n][p])
            bias_sb[n] = t

        # masks resident, loaded first so they overlap stage-1 DMA
        maskpool = ctx.enter_context(tc.tile_pool(name="masks", bufs=1))
        mA = maskpool.tile([128, KC, S], MASK_DT, tag="mA")
        nc.sync.dma_start(out=mA, in_=m01.rearrange("(c p) k -> p c k", p=128))
        mB = maskpool.tile([128, KC, S], MASK_DT, tag="mB")
        nc.sync.dma_start(out=mB, in_=m01T.rearrange("(c p) q -> p c q", p=128))

        # Resident activations
        resident = ctx.enter_context(tc.tile_pool(name="resident", bufs=1))
        QT = resident.tile([128, 2, S], F32R, tag="QT")
        KT = resident.tile([128, 2, S], F32R, tag="KT")
        ctxAll = resident.tile([128, 2, S], F32R, tag="ctxAll")
        vaug = [
            resident.tile([128, KC, DK + 1], BF16, tag=f"vaug{h}", name=f"vaug{h}") for h in range(HPC)
        ]
        negln = [
            resident.tile([128, KC], F32, tag=f"negln{h}", name=f"negln{h}") for h in range(HPC)
        ]

        # ---- Stage 1: projections QT/KT/VT = W @ xT (+bias) ----
        with tc.tile_pool(name="vt", bufs=1) as vtpool:
            VT = vtpool.tile([128, 2, S], F32, tag="VT")
            dests = {"q": QT, "k": KT, "v": VT}
            with (
                tc.tile_pool(name="w", bufs=1) as wpool,
                tc.tile_pool(name="x", bufs=2) as xpool,
                tc.tile_pool(name="psproj", bufs=2, space="PSUM") as psproj,
            ):
                w_sb = {}
                for n in ("q", "k", "v"):
                    w_sb[n] = wpool.tile([128, 8, DC], F32R, tag=f"w{n}", name=f"w{n}")
                    nc.sync.dma_start(
                        out=w_sb[n], in_=wT[n].rearrange("(c p) n -> p c n", p=128)
                    )
                for n in ("q", "k", "v"):  # q,k first: lets pass-B S^T/exp start early
                    ps = [psproj.tile([128, S], F32, tag="proj", name=f"psproj{n}{i}") for i in range(2)]
                    for dm in range(8):
                        xt = xpool.tile([128, S], F32R, tag="xt")
                        nc.sync.dma_start(
                            out=xt, in_=xT[n].rearrange("(c p) s -> c p s", p=128)[dm]
                        )
                        for p in range(2):
                            for j in range(4):
                                nc.tensor.matmul(
                                    ps[p][:, 512 * j : 512 * (j + 1)],
                                    w_sb[n][:, dm, 128 * p : 128 * (p + 1)],
                                    xt[:, 512 * j : 512 * (j + 1)],
                                    start=(dm == 0),
                                    stop=(dm == 7),
                                )
                    for p in range(2):
                        nc.vector.tensor_scalar_add(
                            dests[n][:, p, :], ps[p], bias_sb[n][:, p : p + 1]
                        )

            # ---- Stage 2: V_aug (V natural layout, bf16, +ones col) ----
            with tc.tile_pool(name="psv", bufs=4, space="PSUM") as psv:
                for h in range(HPC):
                    p, rr = h // 2, 64 * (h % 2)
                    nc.gpsimd.memset(vaug[h][:, :, DK : DK + 1], 1.0)
                    for c in range(KC):
                        pst = psv.tile([128, DK], F32, tag="vtr")
                        nc.tensor.transpose(
                            pst,
                            VT[rr : rr + 64, p, 128 * c : 128 * (c + 1)],
                            ident[rr : rr + 64, rr : rr + 64],
                        )
                        nc.vector.tensor_copy(vaug[h][:, c, 0:DK], pst)

        # ---- Attention, chunk-interleaved: round h emits pass-B chunks of
        # head h (S^T -> exp -> mask -> ctx^T accum) alternating with pass-A
        # chunks of head h-1 (S -> exp(bias=-ln den) -> mask -> DMA out), so
        # ACT/DMA/DVE/GPSIMD all stay busy. PSUM: ctx [65,2048] (4 banks) +
        # shared scores pool 2x[128,1024] (4 banks) = 8.
        with (
            tc.tile_pool(name="expb", bufs=4) as expb,
            tc.tile_pool(name="smallb", bufs=2) as smallb,
            tc.tile_pool(name="attn", bufs=4) as attnp,
            tc.tile_pool(name="psctx", bufs=1, space="PSUM") as psctx,
            tc.tile_pool(name="pssc", bufs=3, space="PSUM") as pssc,
        ):
            mult_ctr = 0

            def b_half(h, ha, c, ps_ctx):
                nonlocal mult_ctr
                p, rr = h // 2, 64 * (h % 2)
                q0 = 1024 * ha
                ps_s = pssc.tile(
                    [128, 1024], F32, tag="sc", name=f"psB{h}_{c}_{ha}"
                )
                for j in range(2):
                    nc.tensor.matmul(
                        ps_s[:, 512 * j : 512 * (j + 1)],
                        KT[rr : rr + 64, p, 128 * c : 128 * (c + 1)],
                        QT[rr : rr + 64, p, q0 + 512 * j : q0 + 512 * (j + 1)],
                        start=True,
                        stop=True,
                    )
                et = expb.tile([128, 1024], BF16, tag="exp", name="et")
                nc.scalar.activation(et, ps_s, EXP, scale=float(SCALE))
                em = expb.tile([128, 1024], BF16, tag="expm", name="em")
                mult_ctr += 1
                gpb = GPB0 if h == 0 else GPB
                eng = nc.gpsimd if (mult_ctr % 16) < gpb else nc.vector
                eng.tensor_mul(em, et, mB[:, c, q0 : q0 + 1024])
                for j in range(2):
                    nc.tensor.matmul(
                        ps_ctx[:, 512 * j : 512 * (j + 1)],
                        vaug[h][:, c, :],
                        em[:, 512 * j : 512 * (j + 1)],
                        start=(c == 0),
                        stop=(c == KC - 1),
                    )

            def b_tail_half(h, ha, ps_ctx):
                """Denominators + ctx normalization for head h, q-half ha."""
                p, rr = h // 2, 64 * (h % 2)
                q0 = 1024 * ha
                recip = smallb.tile([1, 1024], F32, tag="recip", name="recip")
                nc.vector.reciprocal(recip, ps_ctx[64:65, :])
                nlog = smallb.tile([1, 1024], F32, tag="nlog", name="nlog")
                nc.scalar.activation(nlog, recip, LN)
                ps_b = pssc.tile([128, 8], F32, tag="sc", name=f"psbias{h}{ha}")
                for qc in range(8):
                    nc.tensor.transpose(
                        ps_b[:, qc : qc + 1],
                        nlog[0:1, 128 * qc : 128 * (qc + 1)],
                        ident[0:1, 0:1],
                    )
                nc.vector.tensor_copy(negln[h][:, 8 * ha : 8 * ha + 8], ps_b)
                ps_rb = pssc.tile([64, 1024], F32, tag="sc", name=f"psrb{h}{ha}")
                for j in range(2):
                    nc.tensor.matmul(
                        ps_rb[:, 512 * j : 512 * (j + 1)],
                        ones64,
                        recip[:, 512 * j : 512 * (j + 1)],
                        start=True,
                        stop=True,
                    )
                rb = smallb.tile([64, 1024], F32, tag="rb", name="rb")
                nc.vector.tensor_copy(rb, ps_rb)
                nc.vector.tensor_mul(
                    ctxAll[rr : rr + 64, p, q0 : q0 + 1024],
                    ps_ctx[0:64, :],
                    rb,
                )

            def a_half(h, qc, ha):
                p, rr = h // 2, 64 * (h % 2)
                k0 = 1024 * ha
                ps_s = pssc.tile(
                    [128, 1024], F32, tag="sc", name=f"psA{h}_{qc}_{ha}"
                )
                for j in range(2):
                    nc.tensor.matmul(
                        ps_s[:, 512 * j : 512 * (j + 1)],
                        QT[rr : rr + 64, p, 128 * qc : 128 * (qc + 1)],
                        KT[rr : rr + 64, p, k0 + 512 * j : k0 + 512 * (j + 1)],
                        start=True,
                        stop=True,
                    )
                ae = attnp.tile([128, 1024], F32, tag="ae", name="ae")
                nc.scalar.activation(
                    ae, ps_s, EXP, scale=float(SCALE),
                    bias=negln[h][:, qc : qc + 1],
                )
                ao = attnp.tile([128, 1024], F32, tag="ao", name="ao")
                nc.vector.tensor_mul(ao, ae, mA[:, qc, k0 : k0 + 1024])
                nc.sync.dma_start(
                    out=attn4[h, 128 * qc : 128 * (qc + 1), k0 : k0 + 1024],
                    in_=ao,
                )

            for h in range(HPC + 1):
                for ha in range(2):
                    ps_ctx = (
                        psctx.tile(
                            [65, 1024], F32, tag="ctx", name=f"ctx{h}{ha}"
                        )
                        if h < HPC
                        else None
                    )
                    for c in range(KC):
                        if h < HPC:
                            b_half(h, ha, c, ps_ctx)
                        if h > 0:
                            i = ha * KC + c
                            a_half(h - 1, i // 2, i % 2)
                    if h < HPC:
                        b_tail_half(h, ha, ps_ctx)

        # ---- Out-projection (partial over this core's 256 ctx dims) ----
        with (
            tc.tile_pool(name="wo", bufs=1) as wop,
            tc.tile_pool(name="outs", bufs=3) as outsp,
            tc.tile_pool(name="pso", bufs=2, space="PSUM") as pso,
        ):
            wo_sb = wop.tile([128, 2, D], F32R, tag="wo")
            nc.sync.dma_start(out=wo_sb, in_=woT.ap())
            for sc in range(KC):
                ps_o = pso.tile([128, D], F32, tag="o")
                for p in range(2):
                    for j in range(2):
                        nc.tensor.matmul(
                            ps_o[:, 512 * j : 512 * (j + 1)],
                            ctxAll[:, p, 128 * sc : 128 * (sc + 1)],
                            wo_sb[:, p, 512 * j : 512 * (j + 1)],
                            start=(p == 0),
                            stop=(p == 1),
                        )
                ot = outsp.tile([128, D], F32, tag="ot")
                nc.vector.tensor_copy(ot, ps_o)
                nc.sync.dma_start(
                    out=outp[128 * sc : 128 * (sc + 1), :], in_=ot
                )

    split_waits(nc)
    return nc


def prep_inputs(query, key, value, mask, Wq, bq, Wk, bk, Wv, bv, Wo, bo):
    """Host-side preprocessing -> list of 8 per-core input maps."""
    mask_np = np.asarray(mask)
    m01_np = [
        np.ascontiguousarray(mask_np[b, 0]).astype(np.uint8) for b in range(B)
    ]
    m01T_np = [np.ascontiguousarray(m.T) for m in m01_np]
    xT_np = {
        "qT": [np.ascontiguousarray(np.asarray(query)[b].T) for b in range(B)],
        "kT": [np.ascontiguousarray(np.asarray(key)[b].T) for b in range(B)],
        "vT": [np.ascontiguousarray(np.asarray(value)[b].T) for b in range(B)],
    }
    W = {"q": np.asarray(Wq), "k": np.asarray(Wk), "v": np.asarray(Wv)}
    bvec = {"q": np.asarray(bq), "k": np.asarray(bk), "v": np.asarray(bv)}
    Wo_np = np.asarray(Wo)
    in_maps = []
    for core in range(NC_CORES):
        b, g = core // 4, core % 4
        rows = slice(DC * g, DC * (g + 1))
        m = {
            "qT": xT_np["qT"][b],
            "kT": xT_np["kT"][b],
            "vT": xT_np["vT"][b],
            "m01": m01_np[b],
            "m01T": m01T_np[b],
            "WoTs": np.ascontiguousarray(
                Wo_np[:, rows].T.reshape(2, 128, D).transpose(1, 0, 2)
            ),
        }
        for n in ("q", "k", "v"):
            m[f"W{n}T"] = np.ascontiguousarray(W[n][rows, :].T)
            m[f"b{n}s"] = np.ascontiguousarray(
                bvec[n][rows].reshape(2, 128, 1).astype(np.float32)
            )
        in_maps.append(m)
    return in_maps


def assemble(results, bo):
    """results: list of 8 per-core output dicts -> (output, attn)."""
    attn = np.empty((B, H, S, S), np.float32)
    output = np.zeros((B, S, D), np.float32)
    for core in range(NC_CORES):
        b, g = core // 4, core % 4
        attn[b, HPC * g : HPC * (g + 1)] = results[core]["attn4"]
        output[b] += results[core]["outp"]
    output += np.asarray(bo)[None, None, :]
    return output, attn




def split_waits(nc, maxw: int = 1) -> int:
    """walrus in this container rejects instructions carrying more than one
    sync-wait; move excess waits onto InstNoOps inserted before the offending
    instruction on the same engine (semantics preserved: the engine blocks on
    them in program order)."""
    n_split = 0
    for fn in nc.m.functions:
        for bb in fn.blocks:
            new_insts = []
            for inst in bb.instructions:
                si = inst.sync_info
                if si is not None and si.on_wait and len(si.on_wait) > maxw:
                    waits = list(si.on_wait)
                    extra, keep = waits[:-maxw], waits[-maxw:]
                    for ci in range(0, len(extra), maxw):
                        nop = mybir.InstNoOp(name=f"I-waitsplit-{nc.next_id()}")
                        nop.engine = inst.engine
                        nop.sync_info = mybir.SyncInfo(
                            on_wait=extra[ci : ci + maxw], on_update=[]
                        )
                        nc.register_instruction(nop)
                        new_insts.append(nop)
                        n_split += 1
                    si.on_wait = keep
                new_insts.append(inst)
            bb.instructions[:] = new_insts
    return n_split


class SpmdRunner:
    """Compile + run the Bass program on n_cores via PJRT/axon, mirroring
    concourse.bass2jax.run_bass_via_pjrt's multi-core path but keeping the
    jitted executable and device-resident inputs for repeated timing."""

    def __init__(self, nc, n_cores=8):
        import jax
        from jax.sharding import Mesh, PartitionSpec
        from jax.experimental.shard_map import shard_map
        from concourse.bass2jax import _bass_exec_p, install_neuronx_cc_hook

        install_neuronx_cc_hook()
        self.jax = jax
        self.nc = nc
        self.n_cores = n_cores
        in_names, out_names, out_avals = [], [], []
        for alloc in nc.m.functions[0].allocations:
            if not isinstance(alloc, mybir.MemoryLocationSet):
                continue
            name = alloc.memorylocations[0].name
            if alloc.kind == "ExternalInput":
                in_names.append(name)
            elif alloc.kind == "ExternalOutput":
                out_names.append(name)
                out_avals.append(
                    jax.core.ShapedArray(
                        tuple(alloc.tensor_shape), mybir.dt.np(alloc.dtype)
                    )
                )
        self.n_params = len(in_names)
        self.out_names = out_names
        self.out_avals = out_avals
        self.in_names = in_names + out_names
        n_outs = len(out_names)
        donate = tuple(range(self.n_params, self.n_params + n_outs))
        all_names = tuple(self.in_names)

        def _body(*args):
            return tuple(
                _bass_exec_p.bind(
                    *args,
                    out_avals=tuple(out_avals),
                    in_names=all_names,
                    out_names=tuple(out_names),
                    lowering_input_output_aliases=(),
                    sim_require_finite=True,
                    sim_require_nnan=True,
                    nc=nc,
                )
            )

        devices = jax.devices()[:n_cores]
        assert len(devices) == n_cores
        self.mesh = Mesh(np.asarray(devices), ("core",))
        in_specs = (PartitionSpec("core"),) * (self.n_params + n_outs)
        out_specs = (PartitionSpec("core"),) * n_outs
        self.sharded = jax.jit(
            shard_map(
                _body,
                mesh=self.mesh,
                in_specs=in_specs,
                out_specs=out_specs,
                check_rep=False,
            ),
            donate_argnums=donate,
            keep_unused=True,
        )
        self._sharding = jax.sharding.NamedSharding(
            self.mesh, PartitionSpec("core")
        )

    def device_inputs(self, in_maps):
        assert len(in_maps) == self.n_cores
        concat = [
            np.ascontiguousarray(
                np.concatenate([np.asarray(m[name]) for m in in_maps], axis=0)
            )
            for name in self.in_names[: self.n_params]
        ]
        return [self.jax.device_put(a, self._sharding) for a in concat]

    def make_zero_outs(self):
        return [
            self.jax.device_put(
                np.zeros((self.n_cores * a.shape[0], *a.shape[1:]), a.dtype),
                self._sharding,
            )
            for a in self.out_avals
        ]

    def split_outs(self, outs):
        return [
            {
                name: np.asarray(outs[i]).reshape(
                    self.n_cores, *self.out_avals[i].shape
                )[c]
                for i, name in enumerate(self.out_names)
            }
            for c in range(self.n_cores)
        ]

    def run(self, dev_in):
        outs = self.sharded(*dev_in, *self.make_zero_outs())
        self.jax.block_until_ready(outs)
        return self.split_outs(outs)

    def _run_pipelined(self, dev_in, n, depth=2):
        import time

        slots = [self.make_zero_outs() for _ in range(depth)]
        self.jax.block_until_ready(slots)
        t0 = time.perf_counter()
        for i in range(n):
            slots[i % depth] = self.sharded(*dev_in, *slots[i % depth])
        self.jax.block_until_ready(slots)
        return time.perf_counter() - t0, slots[(n - 1) % depth]

    def bench_slope(self, dev_in, n_lo=16, n_hi=96, reps=6):
        """Per-exec device time from the wall-time slope between n_lo and
        n_hi pipelined executions (tunnel latency + noise cancel via min)."""
        self._run_pipelined(dev_in, 2)
        lo, hi = [], []
        outs = None
        for _ in range(reps):
            lo.append(self._run_pipelined(dev_in, n_lo)[0])
            t, outs = self._run_pipelined(dev_in, n_hi)
            hi.append(t)
        per_exec = (min(hi) - min(lo)) / (n_hi - n_lo)
        return per_exec, self.split_outs(outs)


_CACHE = {}


def kernel(**inputs):
    if "runner" not in _CACHE:
        nc = build_program()
        _CACHE["runner"] = SpmdRunner(nc, NC_CORES)
    runner = _CACHE["runner"]
    in_maps = prep_inputs(**inputs)
    dev_in = runner.device_inputs(in_maps)
    results = runner.run(dev_in)
    return assemble(results, inputs["bo"])


if __name__ == "__main__":
    nc = build_program()
    print(
        "built ok; instructions:",
        sum(len(bb.instructions) for f in nc.m.functions for bb in f.blocks),
    )


# revision 22
# speedup vs baseline: 2.4304x; 1.1610x over previous
"""Multi-head attention (B=2, S=2048, D=1024, H=16) on 8 trn2 NeuronCores.

Sharding: core = (batch b, head-group g of 4 heads); 2 batches x 4 groups.
Each core computes, for its batch and its 4 heads:
  - transposed projections QT/KT/VT = W @ x^T (fp32r matmuls),
  - pass B ([k,q] orientation): S^T = K Q^T, exp, mask, attn^T @ V via
    V augmented with a ones-row so the softmax denominator falls out of
    the same matmul; context normalized on-chip,
  - pass A ([q,k] orientation): S = Q K^T, exp with bias=-ln(den) per
    partition (q) -> normalized attention probabilities, masked, DMA'd out,
  - partial out-projection for its 256 context dims (host sums over g).
Host: input/weight transposes, mask conversion, output assembly.
"""

import os
import numpy as np
from contextlib import ExitStack

import concourse.bass as bass
import concourse.tile as tile
import concourse.mybir as mybir
from concourse.masks import make_identity

F32 = mybir.dt.float32
F32R = mybir.dt.float32r
BF16 = mybir.dt.bfloat16
U8 = mybir.dt.uint8
EXP = mybir.ActivationFunctionType.Exp
LN = mybir.ActivationFunctionType.Ln

B, S, D = 2, 2048, 1024
H = 16
DK = 64  # head dim
HPC = 4  # heads per core
DC = HPC * DK  # context dims per core (256)
NC_CORES = 8
KC = S // 128  # 16 k/q chunks
SCALE = 1.0 / np.sqrt(DK)

MASK_DT = U8  # mask storage dtype on device
GPB = 10  # of 16 pass-B mask-mults per round, how many on GPSIMD (rest DVE)
GPB0 = 6  # same, for round 0 (DVE idle there)




def build_program():
    nc = bass.Bass(
        "TRN2", target_bir_lowering=False, debug=False, enable_partition_id=False
    )
    # --- DRAM tensors (per-core views; same program on all 8 cores) ---
    xT = {
        n: nc.dram_tensor(f"{n}T", [D, S], F32R, kind="ExternalInput")
        for n in ("q", "k", "v")
    }
    wT = {
        n: nc.dram_tensor(f"W{n}T", [D, DC], F32R, kind="ExternalInput")
        for n in ("q", "k", "v")
    }
    bias_d = {
        n: nc.dram_tensor(f"b{n}s", [2, 128, 1], F32, kind="ExternalInput")
        for n in ("q", "k", "v")
    }
    woT = nc.dram_tensor("WoTs", [128, 2, D], F32R, kind="ExternalInput")
    m01 = nc.dram_tensor("m01", [S, S], MASK_DT, kind="ExternalInput")
    m01T = nc.dram_tensor("m01T", [S, S], MASK_DT, kind="ExternalInput")
    attn4 = nc.dram_tensor("attn4", [HPC, S, S], F32, kind="ExternalOutput")
    outp = nc.dram_tensor("outp", [S, D], F32, kind="ExternalOutput")

    with ExitStack() as ctx:
        tc = ctx.enter_context(tile.TileContext(nc))

        consts = ctx.enter_context(tc.tile_pool(name="consts", bufs=1))
        ident = consts.tile([128, 128], F32)
        make_identity(nc, ident)
        ones64 = consts.tile([1, 64], F32)
        nc.gpsimd.memset(ones64, 1.0)
        bias_sb = {}
        for n in ("q", "k", "v"):
            t = consts.tile([128, 2], F32, tag=f"bias{n}", name=f"bias{n}")
            for p in range(2):
                nc.sync.dma_start(out=t[:, p : p + 1], in_=bias_d[n][p])
            bias_sb[n] = t

        # masks resident, loaded first so they overlap stage-1 DMA
        maskpool = ctx.enter_context(tc.tile_pool(name="masks", bufs=1))
        mA = maskpool.tile([128, KC, S], MASK_DT, tag="mA")
        nc.sync.dma_start(out=mA, in_=m01.rearrange("(c p) k -> p c k", p=128))
        mB = maskpool.tile([128, KC, S], MASK_DT, tag="mB")
        nc.sync.dma_start(out=mB, in_=m01T.rearrange("(c p) q -> p c q", p=128))

        # Resident activations
        resident = ctx.enter_context(tc.tile_pool(name="resident", bufs=1))
        QT = resident.tile([128, 2, S], F32R, tag="QT")
        KT = resident.tile([128, 2, S], F32R, tag="KT")
        ctxAll = resident.tile([128, 2, S], F32R, tag="ctxAll")
        vaug = [
            resident.tile([128, KC, DK + 1], BF16, tag=f"vaug{h}", name=f"vaug{h}") for h in range(HPC)
        ]
        negln = [
            resident.tile([128, KC], F32, tag=f"negln{h}", name=f"negln{h}") for h in range(HPC)
        ]

        # ---- Stage 1: projections QT/KT/VT = W @ xT (+bias) ----
        with tc.tile_pool(name="vt", bufs=1) as vtpool:
            VT = vtpool.tile([128, 2, S], F32, tag="VT")
            dests = {"q": QT, "k": KT, "v": VT}
            with (
                tc.tile_pool(name="w", bufs=1) as wpool,
                tc.tile_pool(name="x", bufs=2) as xpool,
                tc.tile_pool(name="psproj", bufs=2, space="PSUM") as psproj,
            ):
                w_sb = {}
                for n in ("q", "k", "v"):
                    w_sb[n] = wpool.tile([128, 8, DC], F32R, tag=f"w{n}", name=f"w{n}")
                    nc.sync.dma_start(
                        out=w_sb[n], in_=wT[n].rearrange("(c p) n -> p c n", p=128)
                    )
                for n in ("q", "k", "v"):  # q,k first: lets pass-B S^T/exp start early
                    ps = [psproj.tile([128, S], F32, tag="proj", name=f"psproj{n}{i}") for i in range(2)]
                    for dm in range(8):
                        xt = xpool.tile([128, S], F32R, tag="xt")
                        nc.sync.dma_start(
                            out=xt, in_=xT[n].rearrange("(c p) s -> c p s", p=128)[dm]
                        )
                        for p in range(2):
                            for j in range(4):
                                nc.tensor.matmul(
                                    ps[p][:, 512 * j : 512 * (j + 1)],
                                    w_sb[n][:, dm, 128 * p : 128 * (p + 1)],
                                    xt[:, 512 * j : 512 * (j + 1)],
                                    start=(dm == 0),
                                    stop=(dm == 7),
                                )
                    for p in range(2):
                        nc.vector.tensor_scalar_add(
                            dests[n][:, p, :], ps[p], bias_sb[n][:, p : p + 1]
                        )

            # ---- Stage 2: V_aug (V natural layout, bf16, +ones col) ----
            with tc.tile_pool(name="psv", bufs=4, space="PSUM") as psv:
                for h in range(HPC):
                    p, rr = h // 2, 64 * (h % 2)
                    nc.gpsimd.memset(vaug[h][:, :, DK : DK + 1], 1.0)
                    for c in range(KC):
                        pst = psv.tile([128, DK], F32, tag="vtr")
                        nc.tensor.transpose(
                            pst,
                            VT[rr : rr + 64, p, 128 * c : 128 * (c + 1)],
                            ident[rr : rr + 64, rr : rr + 64],
                        )
                        nc.vector.tensor_copy(vaug[h][:, c, 0:DK], pst)

        # ---- Attention, chunk-interleaved: round h emits pass-B chunks of
        # head h (S^T -> exp -> mask -> ctx^T accum) alternating with pass-A
        # chunks of head h-1 (S -> exp(bias=-ln den) -> mask -> DMA out), so
        # ACT/DMA/DVE/GPSIMD all stay busy. PSUM: ctx [65,2048] (4 banks) +
        # shared scores pool 2x[128,1024] (4 banks) = 8.
        with (
            tc.tile_pool(name="expb", bufs=4) as expb,
            tc.tile_pool(name="smallb", bufs=2) as smallb,
            tc.tile_pool(name="attn", bufs=4) as attnp,
            tc.tile_pool(name="psctx", bufs=1, space="PSUM") as psctx,
            tc.tile_pool(name="pssc", bufs=3, space="PSUM") as pssc,
        ):
            mult_ctr = 0

            def b_half(h, ha, c, ps_ctx):
                nonlocal mult_ctr
                p, rr = h // 2, 64 * (h % 2)
                q0 = 1024 * ha
                ps_s = pssc.tile(
                    [128, 1024], F32, tag="sc", name=f"psB{h}_{c}_{ha}"
                )
                for j in range(2):
                    nc.tensor.matmul(
                        ps_s[:, 512 * j : 512 * (j + 1)],
                        KT[rr : rr + 64, p, 128 * c : 128 * (c + 1)],
                        QT[rr : rr + 64, p, q0 + 512 * j : q0 + 512 * (j + 1)],
                        start=True,
                        stop=True,
                    )
                et = expb.tile([128, 1024], BF16, tag="exp", name="et")
                nc.scalar.activation(et, ps_s, EXP, scale=float(SCALE))
                em = expb.tile([128, 1024], BF16, tag="expm", name="em")
                mult_ctr += 1
                gpb = GPB0 if h == 0 else GPB
                eng = nc.gpsimd if (mult_ctr % 16) < gpb else nc.vector
                eng.tensor_mul(em, et, mB[:, c, q0 : q0 + 1024])
                for j in range(2):
                    nc.tensor.matmul(
                        ps_ctx[:, 512 * j : 512 * (j + 1)],
                        vaug[h][:, c, :],
                        em[:, 512 * j : 512 * (j + 1)],
                        start=(c == 0),
                        stop=(c == KC - 1),
                    )

            def b_tail_half(h, ha, ps_ctx):
                """Denominators + ctx normalization for head h, q-half ha."""
                p, rr = h // 2, 64 * (h % 2)
                q0 = 1024 * ha
                recip = smallb.tile([1, 1024], F32, tag="recip", name="recip")
                nc.vector.reciprocal(recip, ps_ctx[64:65, :])
                nlog = smallb.tile([1, 1024], F32, tag="nlog", name="nlog")
                nc.scalar.activation(nlog, recip, LN)
                ps_b = pssc.tile([128, 8], F32, tag="sc", name=f"psbias{h}{ha}")
                for qc in range(8):
                    nc.tensor.transpose(
                        ps_b[:, qc : qc + 1],
                        nlog[0:1, 128 * qc : 128 * (qc + 1)],
                        ident[0:1, 0:1],
                    )
                nc.vector.tensor_copy(negln[h][:, 8 * ha : 8 * ha + 8], ps_b)
                ps_rb = pssc.tile([64, 1024], F32, tag="sc", name=f"psrb{h}{ha}")
                for j in range(2):
                    nc.tensor.matmul(
                        ps_rb[:, 512 * j : 512 * (j + 1)],
                        ones64,
                        recip[:, 512 * j : 512 * (j + 1)],
                        start=True,
                        stop=True,
                    )
                rb = smallb.tile([64, 1024], F32, tag="rb", name="rb")
                nc.vector.tensor_copy(rb, ps_rb)
                nc.vector.tensor_mul(
                    ctxAll[rr : rr + 64, p, q0 : q0 + 1024],
                    ps_ctx[0:64, :],
                    rb,
                )

            def a_half(h, qc, ha):
                p, rr = h // 2, 64 * (h % 2)
                k0 = 1024 * ha
                ps_s = pssc.tile(
                    [128, 1024], F32, tag="sc", name=f"psA{h}_{qc}_{ha}"
                )
                for j in range(2):
                    nc.tensor.matmul(
                        ps_s[:, 512 * j : 512 * (j + 1)],
                        QT[rr : rr + 64, p, 128 * qc : 128 * (qc + 1)],
                        KT[rr : rr + 64, p, k0 + 512 * j : k0 + 512 * (j + 1)],
                        start=True,
                        stop=True,
                    )
                ae = attnp.tile([128, 1024], F32, tag="ae", name="ae")
                nc.scalar.activation(
                    ae, ps_s, EXP, scale=float(SCALE),
                    bias=negln[h][:, qc : qc + 1],
                )
                ao = attnp.tile([128, 1024], F32, tag="ao", name="ao")
                nc.vector.tensor_mul(ao, ae, mA[:, qc, k0 : k0 + 1024])
                nc.sync.dma_start(
                    out=attn4[h, 128 * qc : 128 * (qc + 1), k0 : k0 + 1024],
                    in_=ao,
                )

            for h in range(HPC + 1):
                for ha in range(2):
                    ps_ctx = (
                        psctx.tile(
                            [65, 1024], F32, tag="ctx", name=f"ctx{h}{ha}"
                        )
                        if h < HPC
                        else None
                    )
                    for c in range(KC):
                        if h < HPC:
                            b_half(h, ha, c, ps_ctx)
                        if h > 0:
                            i = ha * KC + c
                            a_half(h - 1, i // 2, i % 2)
                    if h < HPC:
                        b_tail_half(h, ha, ps_ctx)

        # ---- Out-projection (partial over this core's 256 ctx dims) ----
        with (
            tc.tile_pool(name="wo", bufs=1) as wop,
            tc.tile_pool(name="outs", bufs=3) as outsp,
            tc.tile_pool(name="pso", bufs=2, space="PSUM") as pso,
        ):
            wo_sb = wop.tile([128, 2, D], F32R, tag="wo")
            nc.sync.dma_start(out=wo_sb, in_=woT.ap())
            for sc in range(KC):
                ps_o = pso.tile([128, D], F32, tag="o")
                for p in range(2):
                    for j in range(2):
                        nc.tensor.matmul(
                            ps_o[:, 512 * j : 512 * (j + 1)],
                            ctxAll[:, p, 128 * sc : 128 * (sc + 1)],
                            wo_sb[:, p, 512 * j : 512 * (j + 1)],
                            start=(p == 0),
                            stop=(p == 1),
                        )
                ot = outsp.tile([128, D], F32, tag="ot")
                nc.vector.tensor_copy(ot, ps_o)
                nc.sync.dma_start(
                    out=outp[128 * sc : 128 * (sc + 1), :], in_=ot
                )

    split_waits(nc)
    return nc


def prep_inputs(query, key, value, mask, Wq, bq, Wk, bk, Wv, bv, Wo, bo):
    """Host-side preprocessing -> list of 8 per-core input maps."""
    mask_np = np.asarray(mask)
    m01_np = [
        np.ascontiguousarray(mask_np[b, 0]).astype(np.uint8) for b in range(B)
    ]
    m01T_np = [np.ascontiguousarray(m.T) for m in m01_np]
    xT_np = {
        "qT": [np.ascontiguousarray(np.asarray(query)[b].T) for b in range(B)],
        "kT": [np.ascontiguousarray(np.asarray(key)[b].T) for b in range(B)],
        "vT": [np.ascontiguousarray(np.asarray(value)[b].T) for b in range(B)],
    }
    W = {"q": np.asarray(Wq), "k": np.asarray(Wk), "v": np.asarray(Wv)}
    bvec = {"q": np.asarray(bq), "k": np.asarray(bk), "v": np.asarray(bv)}
    Wo_np = np.asarray(Wo)
    in_maps = []
    for core in range(NC_CORES):
        b, g = core // 4, core % 4
        rows = slice(DC * g, DC * (g + 1))
        m = {
            "qT": xT_np["qT"][b],
            "kT": xT_np["kT"][b],
            "vT": xT_np["vT"][b],
            "m01": m01_np[b],
            "m01T": m01T_np[b],
            "WoTs": np.ascontiguousarray(
                Wo_np[:, rows].T.reshape(2, 128, D).transpose(1, 0, 2)
            ),
        }
        for n in ("q", "k", "v"):
            m[f"W{n}T"] = np.ascontiguousarray(W[n][rows, :].T)
            m[f"b{n}s"] = np.ascontiguousarray(
                bvec[n][rows].reshape(2, 128, 1).astype(np.float32)
            )
        in_maps.append(m)
    return in_maps


def assemble(results, bo):
    """results: list of 8 per-core output dicts -> (output, attn)."""
    attn = np.empty((B, H, S, S), np.float32)
    output = np.zeros((B, S, D), np.float32)
    for core in range(NC_CORES):
        b, g = core // 4, core % 4
        attn[b, HPC * g : HPC * (g + 1)] = results[core]["attn4"]
        output[b] += results[core]["outp"]
    output += np.asarray(bo)[None, None, :]
    return output, attn




def split_waits(nc, maxw: int = 1) -> int:
    """walrus in this container rejects instructions carrying more than one
    sync-wait; move excess waits onto InstNoOps inserted before the offending
    instruction on the same engine (semantics preserved: the engine blocks on
    them in program order)."""
    n_split = 0
    for fn in nc.m.functions:
        for bb in fn.blocks:
            new_insts = []
            for inst in bb.instructions:
                si = inst.sync_info
                if si is not None and si.on_wait and len(si.on_wait) > maxw:
                    waits = list(si.on_wait)
                    extra, keep = waits[:-maxw], waits[-maxw:]
                    for ci in range(0, len(extra), maxw):
                        nop = mybir.InstNoOp(name=f"I-waitsplit-{nc.next_id()}")
                        nop.engine = inst.engine
                        nop.sync_info = mybir.SyncInfo(
                            on_wait=extra[ci : ci + maxw], on_update=[]
                        )
                        nc.register_instruction(nop)
                        new_insts.append(nop)
                        n_split += 1
                    si.on_wait = keep
                new_insts.append(inst)
            bb.instructions[:] = new_insts
    return n_split


class SpmdRunner:
    """Compile + run the Bass program on n_cores via PJRT/axon, mirroring
    concourse.bass2jax.run_bass_via_pjrt's multi-core path but keeping the
    jitted executable and device-resident inputs for repeated timing."""

    def __init__(self, nc, n_cores=8):
        import jax
        from jax.sharding import Mesh, PartitionSpec
        from jax.experimental.shard_map import shard_map
        from concourse.bass2jax import _bass_exec_p, install_neuronx_cc_hook

        install_neuronx_cc_hook()
        self.jax = jax
        self.nc = nc
        self.n_cores = n_cores
        in_names, out_names, out_avals = [], [], []
        for alloc in nc.m.functions[0].allocations:
            if not isinstance(alloc, mybir.MemoryLocationSet):
                continue
            name = alloc.memorylocations[0].name
            if alloc.kind == "ExternalInput":
                in_names.append(name)
            elif alloc.kind == "ExternalOutput":
                out_names.append(name)
                out_avals.append(
                    jax.core.ShapedArray(
                        tuple(alloc.tensor_shape), mybir.dt.np(alloc.dtype)
                    )
                )
        self.n_params = len(in_names)
        self.out_names = out_names
        self.out_avals = out_avals
        self.in_names = in_names + out_names
        n_outs = len(out_names)
        donate = tuple(range(self.n_params, self.n_params + n_outs))
        all_names = tuple(self.in_names)

        def _body(*args):
            return tuple(
                _bass_exec_p.bind(
                    *args,
                    out_avals=tuple(out_avals),
                    in_names=all_names,
                    out_names=tuple(out_names),
                    lowering_input_output_aliases=(),
                    sim_require_finite=True,
                    sim_require_nnan=True,
                    nc=nc,
                )
            )

        devices = jax.devices()[:n_cores]
        assert len(devices) == n_cores
        self.mesh = Mesh(np.asarray(devices), ("core",))
        in_specs = (PartitionSpec("core"),) * (self.n_params + n_outs)
        out_specs = (PartitionSpec("core"),) * n_outs
        self.sharded = jax.jit(
            shard_map(
                _body,
                mesh=self.mesh,
                in_specs=in_specs,
                out_specs=out_specs,
                check_rep=False,
            ),
            donate_argnums=donate,
            keep_unused=True,
        )
        self._sharding = jax.sharding.NamedSharding(
            self.mesh, PartitionSpec("core")
        )

    def device_inputs(self, in_maps):
        assert len(in_maps) == self.n_cores
        concat = [
            np.ascontiguousarray(
                np.concatenate([np.asarray(m[name]) for m in in_maps], axis=0)
            )
            for name in self.in_names[: self.n_params]
        ]
        return [self.jax.device_put(a, self._sharding) for a in concat]

    def make_zero_outs(self):
        return [
            self.jax.device_put(
                np.zeros((self.n_cores * a.shape[0], *a.shape[1:]), a.dtype),
                self._sharding,
            )
            for a in self.out_avals
        ]

    def split_outs(self, outs):
        return [
            {
                name: np.asarray(outs[i]).reshape(
                    self.n_cores, *self.out_avals[i].shape
                )[c]
                for i, name in enumerate(self.out_names)
            }
            for c in range(self.n_cores)
        ]

    def run(self, dev_in):
        outs = self.sharded(*dev_in, *self.make_zero_outs())
        self.jax.block_until_ready(outs)
        return self.split_outs(outs)

    def _run_pipelined(self, dev_in, n, depth=2):
        import time

        slots = [self.make_zero_outs() for _ in range(depth)]
        self.jax.block_until_ready(slots)
        t0 = time.perf_counter()
        for i in range(n):
            slots[i % depth] = self.sharded(*dev_in, *slots[i % depth])
        self.jax.block_until_ready(slots)
        return time.perf_counter() - t0, slots[(n - 1) % depth]

    def bench_slope(self, dev_in, n_lo=16, n_hi=96, reps=6):
        """Per-exec device time from the wall-time slope between n_lo and
        n_hi pipelined executions (tunnel latency + noise cancel via min)."""
        self._run_pipelined(dev_in, 2)
        lo, hi = [], []
        outs = None
        for _ in range(reps):
            lo.append(self._run_pipelined(dev_in, n_lo)[0])
            t, outs = self._run_pipelined(dev_in, n_hi)
            hi.append(t)
        per_exec = (min(hi) - min(lo)) / (n_hi - n_lo)
        return per_exec, self.split_outs(outs)


_CACHE = {}


def kernel(**inputs):
    if "runner" not in _CACHE:
        nc = build_program()
        _CACHE["runner"] = SpmdRunner(nc, NC_CORES)
    runner = _CACHE["runner"]
    in_maps = prep_inputs(**inputs)
    dev_in = runner.device_inputs(in_maps)
    results = runner.run(dev_in)
    return assemble(results, inputs["bo"])


if __name__ == "__main__":
    nc = build_program()
    print(
        "built ok; instructions:",
        sum(len(bb.instructions) for f in nc.m.functions for bb in f.blocks),
    )
